# revision 46
# baseline (speedup 1.0000x reference)
"""Bahdanau attention Trainium2 kernel (8 NeuronCores, data-parallel over batch).

Problem shapes: B=64, S=2048, ENC=DEC=512, ATTN=256 (fp32 inputs).
Strategy:
  - Shard batch across 8 cores (8 batches/core); replicate the small weights.
  - Host pre-casts enc_outputs to bf16 and supplies it in BOTH layouts:
      encT [Bl, E, S]  -> feeds the projection matmul (contract E on partitions)
      encN [Bl, S, E]  -> feeds the context matmul   (contract S on partitions)
    Total HBM read/core = 2 * 16.8MB = 33.6MB ~= the fp32 single-read roofline,
    with zero on-device transpose/cast cost.
  - Per (batch, s-chunk of 512): projT = W_enc^T @ encT (PSUM f32), tanh(+pd bias)
    on ScalarE -> T bf16, score row = v^T @ T (M=1 matmuls col-tiled across the
    4 s-chunks), exp on ScalarE at the col-group lanes, strided-DMA gather into
    a packed wu tile (batch rows at partitions {0..3, 32..35}).
  - Softmax: scores are bounded (|score| <= |v|_1) so no max subtraction needed;
    per batch-half (4 rows), overlapped with the other half's main loop:
    wuT = transpose(wu) * maskT (bf16, unnormalized); Z via PE ones-matmuls
    over wuT; weights = wu * mask * (1/Z);
    context = (wuT^T @ encN) with rows scaled by (1/Z) on the way out of PSUM,
    4 batches concurrently via PE column-group tiling.
"""

import os
import sys

import numpy as np

for _p in ("/opt/trn_rl_repo", "/root/.axon_site/_ro/trn_rl_repo"):
    if os.path.isdir(_p) and _p not in sys.path:
        sys.path.insert(0, _p)

import ml_dtypes

import concourse.bass as bass
import concourse.bacc as bacc
import concourse.mybir as mybir
from concourse import tile
from concourse import bass_utils as _bu
from concourse.bass_utils import run_bass_kernel_spmd


BF16 = ml_dtypes.bfloat16

B, S, E, A = 64, 2048, 512, 256
NCORES = 8
BL = B // NCORES          # batches per core = 8
HB = BL // 2              # half-batch group = 4
SCN = 4                   # s-chunks per row of 512
SCW = S // SCN            # 512
ECH = E // 128            # 4 contraction chunks
ACH = A // 128            # 2 output (attn) chunks
CCH = S // 128            # 16 context contraction chunks

F32 = mybir.dt.float32
BF = mybir.dt.bfloat16

_CACHE = {}
LAST_RESULTS = None

_STAGE = int(os.environ.get("BAHDANAU_STAGE", "5"))


def _row(b):
    """wu/mask row partition for local batch b: halves live at 0..3 / 32..35."""
    return b if b < HB else 32 + (b - HB)


def _emit(nc):
    encT = nc.declare_dram_parameter("encT", [BL, E, S], BF, isOutput=False)
    encN = nc.declare_dram_parameter("encN", [BL, S, E], BF, isOutput=False)
    dsT = nc.declare_dram_parameter("dsT", [E, BL], BF, isOutput=False)
    maskN = nc.declare_dram_parameter("maskN", [BL, S], F32, isOutput=False)
    maskT = nc.declare_dram_parameter("maskT", [128, CCH * BL], F32, isOutput=False)
    Wenc = nc.declare_dram_parameter("Wenc", [E, A], BF, isOutput=False)
    Wdec = nc.declare_dram_parameter("Wdec", [E, A], BF, isOutput=False)
    vT = nc.declare_dram_parameter("vT", [128, ACH], BF, isOutput=False)
    idq = nc.declare_dram_parameter("idq", [36, HB], F32, isOutput=False)
    ctx_out = nc.declare_dram_parameter("ctx", [BL, E], F32, isOutput=True)
    wts_out = nc.declare_dram_parameter("wts", [BL, S], F32, isOutput=True)

    with tile.TileContext(nc) as tc:
        with (
            tc.tile_pool(name="consts", bufs=1) as cp,
            tc.tile_pool(name="encn", bufs=BL) as encn_pool,
            tc.tile_pool(name="et", bufs=4) as et_pool,
            tc.tile_pool(name="tt", bufs=6) as t_pool,
            tc.tile_pool(name="es", bufs=2) as es_pool,
        ):
            # ---- constants / small inputs ----
            wenc_sb = cp.tile([128, ECH * A], BF)      # [:, e*A + a*128 ...]
            wdec_sb = cp.tile([128, ECH * A], BF)
            v_sb = cp.tile([128, ACH], BF)
            ds_sb = cp.tile([128, ECH * BL], BF)
            idq_sb = cp.tile([36, HB], F32)
            mask_sb = cp.tile([36, S], F32)
            maskT_sb = cp.tile([128, CCH * BL], F32)
            pd_sb = cp.tile([128, ACH * BL], F32)
            wu_sb = cp.tile([36, S], F32)
            wuT_sb = cp.tile([128, CCH * BL], BF)
            z_sb = cp.tile([1, BL], F32)
            zi_row = cp.tile([1, BL], F32)
            zi_col = cp.tile([36, 1], F32)
            zi_bcast = cp.tile([128, BL], F32)
            ones_row = cp.tile([1, 128], F32)
            ones_colb = cp.tile([128, 1], BF)
            nc.vector.memset(ones_row[:], 1.0)
            nc.vector.memset(ones_colb[:], 1.0)

            # constants go on the scalar engine's DMA queue so the sync
            # engine's ring starts with the first encT tile
            nc.scalar.dma_start(
                out=wenc_sb[:].rearrange("p (ec a) -> p ec a", ec=ECH),
                in_=Wenc.rearrange("(ec p) a -> p ec a", p=128))
            nc.scalar.dma_start(
                out=wdec_sb[:].rearrange("p (ec a) -> p ec a", ec=ECH),
                in_=Wdec.rearrange("(ec p) a -> p ec a", p=128))
            nc.scalar.dma_start(
                out=ds_sb[:].rearrange("p (ec b) -> p ec b", ec=ECH),
                in_=dsT.rearrange("(ec p) b -> p ec b", p=128))
            nc.scalar.dma_start(out=v_sb[:], in_=vT[:])
            nc.scalar.dma_start(out=idq_sb[:], in_=idq[:])
            nc.scalar.dma_start(out=mask_sb[0:HB, :], in_=maskN[0:HB, :])
            nc.scalar.dma_start(out=mask_sb[32:32 + HB, :], in_=maskN[HB:BL, :])
            nc.scalar.dma_start(out=maskT_sb[:], in_=maskT[:])

            with tc.tile_pool(name="misc_ps", bufs=1, space="PSUM") as mp:
                # proj_dec^T for all local batches: [128 (a-half), BL] x ACH
                pd_ps = mp.tile([128, ACH * BL], F32)
                for a in range(ACH):
                    for e in range(ECH):
                        nc.tensor.matmul(
                            pd_ps[:, a * BL:(a + 1) * BL],
                            wdec_sb[:, e * A + a * 128: e * A + (a + 1) * 128],
                            ds_sb[:, e * BL:(e + 1) * BL],
                            start=(e == 0), stop=(e == ECH - 1),
                        )
                nc.vector.tensor_copy(pd_sb[:], pd_ps[:])

            if _STAGE < 2:
                return

            encn_tiles = [encn_pool.tile([128, CCH * E], BF, tag="encn",
                                         name=f"ent{i}")
                          for i in range(BL)]

            def softmax_half(hb, tp, zp):
                """Transpose+mask this half's wu rows into wuT (bf16), Z via
                PE ones-matmuls, then 1/Z in row/col/broadcast layouts."""
                r0 = 0 if hb == 0 else 32
                z_ps = zp.tile([1, HB], F32, tag="z", name=f"z{hb}")
                for c in range(CCH):
                    tr = tp.tile([128, HB], F32, tag="tr", name=f"tr{hb}_{c}")
                    nc.tensor.transpose(
                        tr[:], wu_sb[r0:r0 + HB, c * 128:(c + 1) * 128],
                        idq_sb[r0:r0 + HB, :])
                    sl = slice(c * BL + hb * HB, c * BL + (hb + 1) * HB)
                    nc.vector.tensor_mul(wuT_sb[:, sl], tr[:], maskT_sb[:, sl])
                    nc.tensor.matmul(
                        z_ps[:], ones_colb[:], wuT_sb[:, sl],
                        start=(c == 0), stop=(c == CCH - 1),
                    )
                zsl = slice(hb * HB, (hb + 1) * HB)
                nc.vector.tensor_copy(z_sb[0:1, zsl], z_ps[:])
                nc.vector.reciprocal(zi_row[0:1, zsl], z_sb[0:1, zsl])
                zb_ps = zp.tile([128, HB], F32, tag="z", name=f"zb{hb}")
                nc.tensor.matmul(zb_ps[:], ones_row[:], zi_row[0:1, zsl],
                                 start=True, stop=True)
                nc.vector.tensor_copy(zi_bcast[:, zsl], zb_ps[:])
                zc_ps = zp.tile([36, 1], F32, tag="z", name=f"zc{hb}")
                nc.tensor.matmul(zc_ps[r0:r0 + HB, :], zi_row[0:1, zsl],
                                 ones_row[0:1, 0:1],
                                 tile_position=(0, r0),
                                 start=True, stop=True)
                nc.vector.tensor_copy(zi_col[r0:r0 + HB, :],
                                      zc_ps[r0:r0 + HB, :])
                # weights output rows for this half: wu * mask * (1/Z)
                nc.vector.tensor_mul(wu_sb[r0:r0 + HB, :], wu_sb[r0:r0 + HB, :],
                                     mask_sb[r0:r0 + HB, :])
                nc.vector.tensor_scalar_mul(wu_sb[r0:r0 + HB, :],
                                            wu_sb[r0:r0 + HB, :],
                                            zi_col[r0:r0 + HB, :])
                nc.scalar.dma_start(out=wts_out[hb * HB:(hb + 1) * HB, :],
                                    in_=wu_sb[r0:r0 + HB, :])

            # ---- main loop: projection + tanh + scores + exp ----
            with (
                tc.tile_pool(name="proj_ps", bufs=4, space="PSUM") as pp,
                tc.tile_pool(name="score_ps", bufs=1, space="PSUM") as sp,
                tc.tile_pool(name="tr_ps", bufs=2, space="PSUM") as tp,
                tc.tile_pool(name="z_ps", bufs=1, space="PSUM") as zp,
            ):
                for b in range(BL):
                    # one score psum tile per batch; the 4 s-chunks land at
                    # partitions {0,32,64,96} via column-group tiling
                    sc_ps = sp.tile([128, SCW], F32, tag="score")
                    wuS = es_pool.tile([128, SCW], F32, tag="wus")
                    for sc in range(SCN):
                        et = et_pool.tile([128, ECH * SCW], BF, tag="et")
                        et_src = encT[b].rearrange("(ec p) s -> p ec s", p=128)[
                            :, :, sc * SCW:(sc + 1) * SCW]
                        et_dst = et[:].rearrange("p (ec s) -> p ec s", ec=ECH)
                        if b == 0 and sc == 0:
                            # fast start: first tile in per-chunk pieces so the
                            # first matmul fires after ~128KB instead of 1MB
                            for e in range(ECH):
                                nc.sync.dma_start(out=et_dst[:, e:e + 1, :],
                                                  in_=et_src[:, e:e + 1, :])
                        else:
                            nc.sync.dma_start(out=et_dst, in_=et_src)

                        for a in range(ACH):
                            proj = pp.tile([128, SCW], F32, tag="proj")
                            for e in range(ECH):
                                nc.tensor.matmul(
                                    proj[:],
                                    wenc_sb[:, e * A + a * 128: e * A + (a + 1) * 128],
                                    et[:, e * SCW:(e + 1) * SCW],
                                    start=(e == 0), stop=(e == ECH - 1),
                                )
                            tt = t_pool.tile([128, SCW], BF, tag="tt")
                            nc.scalar.activation(
                                tt[:], proj[:],
                                mybir.ActivationFunctionType.Tanh,
                                bias=pd_sb[:, a * BL + b: a * BL + b + 1],
                            )
                            nc.tensor.matmul(
                                sc_ps[32 * sc:32 * sc + 1, :],
                                v_sb[:, a:a + 1], tt[:],
                                tile_position=(0, 32 * sc),
                                start=(a == 0), stop=(a == ACH - 1),
                            )
                        nc.scalar.activation(
                            wuS[32 * sc:32 * sc + 1, :],
                            sc_ps[32 * sc:32 * sc + 1, :],
                            mybir.ActivationFunctionType.Exp)
                    # this batch's natural-layout blocks (consumed by the
                    # context matmuls) ride the same FIFO ring AFTER the et
                    # tiles so they can't front-run the latency-critical stream
                    nc.sync.dma_start(
                        out=encn_tiles[b][:].rearrange("p (c e) -> p c e", c=CCH),
                        in_=encN[b].rearrange("(c p) e -> p c e", p=128))
                    # gather the 4 strided rows into the packed wu tile
                    r = _row(b)
                    nc.gpsimd.dma_start(out=wu_sb[r:r + 1, :],
                                        in_=wuS[0:128:32, :])
                    if _STAGE >= 3 and b == HB - 1:
                        softmax_half(0, tp, zp)
                if _STAGE >= 3:
                    softmax_half(1, tp, zp)

            if _STAGE < 5:
                return
            # ---- context matmuls: 4 batches concurrently per column-group ----
            with tc.tile_pool(name="ctx_ps", bufs=2, space="PSUM") as cxp:
                for g in range(2):
                    cx = cxp.tile([128, E], F32, tag="cx")
                    for c in range(CCH):
                        for j in range(4):
                            b = g * HB + j
                            nc.tensor.matmul(
                                cx[32 * j:32 * j + 1, :],
                                wuT_sb[:, c * BL + g * HB + j:
                                       c * BL + g * HB + j + 1],
                                encn_tiles[b][:, c * E:(c + 1) * E],
                                tile_position=(0, 32 * j),
                                start=(c == 0), stop=(c == CCH - 1),
                            )
                    cxs = es_pool.tile([128, E], F32, tag="cxs")
                    for j in range(4):
                        b = g * HB + j
                        nc.vector.tensor_scalar_mul(
                            cxs[32 * j:32 * j + 1, :],
                            cx[32 * j:32 * j + 1, :],
                            zi_bcast[32 * j:32 * j + 1, b:b + 1])
                        nc.scalar.dma_start(out=ctx_out[b:b + 1, :],
                                            in_=cxs[32 * j:32 * j + 1, :])
    return nc


def _get_nc():
    if "nc" not in _CACHE:
        nc = bacc.Bacc()
        _emit(nc)
        nc.finalize()
        _CACHE["nc"] = nc
    return _CACHE["nc"]


def _make_idq():
    idq = np.zeros((36, HB), dtype=np.float32)
    idq[0:HB, :] = np.eye(HB, dtype=np.float32)
    idq[32:32 + HB, :] = np.eye(HB, dtype=np.float32)
    return idq


def kernel(dec_state, enc_outputs, src_mask, W_enc, W_dec, v):
    global LAST_RESULTS
    dec_state = np.asarray(dec_state, dtype=np.float32)
    enc_outputs = np.asarray(enc_outputs, dtype=np.float32)
    src_mask = np.asarray(src_mask)
    W_enc = np.asarray(W_enc, dtype=np.float32)
    W_dec = np.asarray(W_dec, dtype=np.float32)
    v = np.asarray(v, dtype=np.float32)

    encN_full = enc_outputs.astype(BF16)                                  # [B,S,E]
    encT_full = np.ascontiguousarray(enc_outputs.transpose(0, 2, 1)).astype(BF16)
    ds_bf = dec_state.astype(BF16)
    mask_f = src_mask.astype(np.float32)
    We_bf = W_enc.astype(BF16)
    Wd_bf = W_dec.astype(BF16)
    vT = np.ascontiguousarray(v.astype(BF16).reshape(ACH, 128).T)         # [128,ACH]
    idq = _make_idq()

    in_maps = []
    for cid in range(NCORES):
        sl = slice(cid * BL, (cid + 1) * BL)
        mshard = mask_f[sl]                                               # [BL,S]
        mT = np.ascontiguousarray(
            mshard.reshape(BL, CCH, 128).transpose(2, 1, 0).reshape(128, CCH * BL))
        in_maps.append({
            "encT": encT_full[sl],
            "encN": encN_full[sl],
            "dsT": np.ascontiguousarray(ds_bf[sl].T),
            "maskN": mshard,
            "maskT": mT,
            "Wenc": We_bf,
            "Wdec": Wd_bf,
            "vT": vT,
            "idq": idq,
        })

    nc = _get_nc()
    res = run_bass_kernel_spmd(nc, in_maps, core_ids=list(range(NCORES)))
    LAST_RESULTS = res

    ctx = np.concatenate([np.asarray(res.results[c]["ctx"]) for c in range(NCORES)], 0)
    wts = np.concatenate([np.asarray(res.results[c]["wts"]) for c in range(NCORES)], 0)
    return ctx.astype(np.float32), wts.astype(np.float32)


# revision 54
# speedup vs baseline: 1.0065x; 1.0065x over previous
"""Bahdanau attention Trainium2 kernel (8 NeuronCores, data-parallel over batch).

Problem shapes: B=64, S=2048, ENC=DEC=512, ATTN=256 (fp32 inputs).
Strategy:
  - Shard batch across 8 cores (8 batches/core); replicate the small weights.
  - Host pre-casts enc_outputs to bf16 and supplies it in BOTH layouts:
      encT [Bl, E, S]  -> feeds the projection matmul (contract E on partitions)
      encN [Bl, S, E]  -> feeds the context matmul   (contract S on partitions)
    Total HBM read/core = 2 * 16.8MB = 33.6MB ~= the fp32 single-read roofline,
    with zero on-device transpose/cast cost.
  - Per (batch, s-chunk of 512): projT = W_enc^T @ encT (PSUM f32), tanh(+pd bias)
    on ScalarE -> T bf16, score row = v^T @ T (M=1 matmuls col-tiled across the
    4 s-chunks), exp on ScalarE at the col-group lanes, strided-DMA gather into
    a packed wu tile (batch rows at partitions {0..3, 32..35}).
  - Softmax: scores are bounded (|score| <= |v|_1) so no max subtraction needed;
    per batch-half (4 rows), overlapped with the other half's main loop:
    wuT = transpose(wu) * maskT (bf16, unnormalized); Z via PE ones-matmuls
    over wuT; weights = wu * mask * (1/Z);
    context = (wuT^T @ encN) with rows scaled by (1/Z) on the way out of PSUM,
    4 batches concurrently via PE column-group tiling.
"""

import os
import sys

import numpy as np

for _p in ("/opt/trn_rl_repo", "/root/.axon_site/_ro/trn_rl_repo"):
    if os.path.isdir(_p) and _p not in sys.path:
        sys.path.insert(0, _p)

import ml_dtypes

import concourse.bass as bass
import concourse.bacc as bacc
import concourse.mybir as mybir
from concourse import tile
from concourse import bass_utils as _bu
from concourse.bass_utils import run_bass_kernel_spmd


BF16 = ml_dtypes.bfloat16

B, S, E, A = 64, 2048, 512, 256
NCORES = 8
BL = B // NCORES          # batches per core = 8
HB = BL // 2              # half-batch group = 4
SCN = 4                   # s-chunks per row of 512
SCW = S // SCN            # 512
ECH = E // 128            # 4 contraction chunks
ACH = A // 128            # 2 output (attn) chunks
CCH = S // 128            # 16 context contraction chunks

F32 = mybir.dt.float32
BF = mybir.dt.bfloat16

_CACHE = {}
LAST_RESULTS = None

_STAGE = int(os.environ.get("BAHDANAU_STAGE", "5"))


def _row(b):
    """wu/mask row partition for local batch b."""
    return b


def _emit(nc):
    encT = nc.declare_dram_parameter("encT", [BL, E, S], BF, isOutput=False)
    encN = nc.declare_dram_parameter("encN", [BL, S, E], BF, isOutput=False)
    dsT = nc.declare_dram_parameter("dsT", [E, BL], BF, isOutput=False)
    maskN = nc.declare_dram_parameter("maskN", [BL, S], F32, isOutput=False)
    maskT = nc.declare_dram_parameter("maskT", [128, CCH * BL], F32, isOutput=False)
    Wenc = nc.declare_dram_parameter("Wenc", [E, A], BF, isOutput=False)
    Wdec = nc.declare_dram_parameter("Wdec", [E, A], BF, isOutput=False)
    vT = nc.declare_dram_parameter("vT", [128, ACH], BF, isOutput=False)
    idq = nc.declare_dram_parameter("idq", [BL, BL], F32, isOutput=False)
    ctx_out = nc.declare_dram_parameter("ctx", [BL, E], F32, isOutput=True)
    wts_out = nc.declare_dram_parameter("wts", [BL, S], F32, isOutput=True)

    with tile.TileContext(nc) as tc:
        with (
            tc.tile_pool(name="consts", bufs=1) as cp,
            tc.tile_pool(name="encn", bufs=BL) as encn_pool,
            tc.tile_pool(name="et", bufs=4) as et_pool,
            tc.tile_pool(name="tt", bufs=6) as t_pool,
            tc.tile_pool(name="es", bufs=2) as es_pool,
        ):
            # ---- constants / small inputs ----
            wenc_sb = cp.tile([128, ECH * A], BF)      # [:, e*A + a*128 ...]
            wdec_sb = cp.tile([128, ECH * A], BF)
            v_sb = cp.tile([128, ACH], BF)
            ds_sb = cp.tile([128, ECH * BL], BF)
            id8_sb = cp.tile([BL, BL], F32)
            mask_sb = cp.tile([BL, S], F32)
            maskT_sb = cp.tile([128, CCH * BL], F32)
            pd_sb = cp.tile([128, ACH * BL], F32)
            wu_sb = cp.tile([BL, S], F32)
            wuT_sb = cp.tile([128, CCH * BL], BF)
            z_sb = cp.tile([1, BL], F32)
            zi_row = cp.tile([1, BL], F32)
            zi_col = cp.tile([BL, 1], F32)
            zi_bcast = cp.tile([128, BL], F32)
            ones_row = cp.tile([1, 128], F32)
            ones_colb = cp.tile([128, 1], BF)
            nc.vector.memset(ones_row[:], 1.0)
            nc.vector.memset(ones_colb[:], 1.0)

            # constants go on the scalar engine's DMA queue so the sync
            # engine's ring starts with the first encT tile
            nc.scalar.dma_start(
                out=wenc_sb[:].rearrange("p (ec a) -> p ec a", ec=ECH),
                in_=Wenc.rearrange("(ec p) a -> p ec a", p=128))
            nc.scalar.dma_start(
                out=wdec_sb[:].rearrange("p (ec a) -> p ec a", ec=ECH),
                in_=Wdec.rearrange("(ec p) a -> p ec a", p=128))
            nc.scalar.dma_start(
                out=ds_sb[:].rearrange("p (ec b) -> p ec b", ec=ECH),
                in_=dsT.rearrange("(ec p) b -> p ec b", p=128))
            nc.scalar.dma_start(out=v_sb[:], in_=vT[:])
            nc.scalar.dma_start(out=id8_sb[:], in_=idq[:])
            nc.scalar.dma_start(out=mask_sb[:], in_=maskN[:])
            nc.scalar.dma_start(out=maskT_sb[:], in_=maskT[:])

            with tc.tile_pool(name="misc_ps", bufs=1, space="PSUM") as mp:
                # proj_dec^T for all local batches: [128 (a-half), BL] x ACH
                pd_ps = mp.tile([128, ACH * BL], F32)
                for a in range(ACH):
                    for e in range(ECH):
                        nc.tensor.matmul(
                            pd_ps[:, a * BL:(a + 1) * BL],
                            wdec_sb[:, e * A + a * 128: e * A + (a + 1) * 128],
                            ds_sb[:, e * BL:(e + 1) * BL],
                            start=(e == 0), stop=(e == ECH - 1),
                        )
                nc.vector.tensor_copy(pd_sb[:], pd_ps[:])

            if _STAGE < 2:
                return

            encn_tiles = [encn_pool.tile([128, CCH * E], BF, tag="encn",
                                         name=f"ent{i}")
                          for i in range(BL)]

            # ---- main loop: projection + tanh + scores + exp ----
            with (
                tc.tile_pool(name="proj_ps", bufs=6, space="PSUM") as pp,
                tc.tile_pool(name="score_ps", bufs=2, space="PSUM") as sp,
            ):
                for b in range(BL):
                    # one score psum tile per batch; the 4 s-chunks land at
                    # partitions {0,32,64,96} via column-group tiling
                    sc_ps = sp.tile([128, SCW], F32, tag="score")
                    wuS = es_pool.tile([128, SCW], F32, tag="wus")
                    for sc in range(SCN):
                        et = et_pool.tile([128, ECH * SCW], BF, tag="et")
                        et_src = encT[b].rearrange("(ec p) s -> p ec s", p=128)[
                            :, :, sc * SCW:(sc + 1) * SCW]
                        et_dst = et[:].rearrange("p (ec s) -> p ec s", ec=ECH)
                        if b == 0 and sc == 0:
                            # fast start: first tile in per-chunk pieces so the
                            # first matmul fires after ~128KB instead of 1MB
                            for e in range(ECH):
                                nc.sync.dma_start(out=et_dst[:, e:e + 1, :],
                                                  in_=et_src[:, e:e + 1, :])
                        else:
                            nc.sync.dma_start(out=et_dst, in_=et_src)

                        for a in range(ACH):
                            proj = pp.tile([128, SCW], F32, tag="proj")
                            for e in range(ECH):
                                nc.tensor.matmul(
                                    proj[:],
                                    wenc_sb[:, e * A + a * 128: e * A + (a + 1) * 128],
                                    et[:, e * SCW:(e + 1) * SCW],
                                    start=(e == 0), stop=(e == ECH - 1),
                                )
                            tt = t_pool.tile([128, SCW], BF, tag="tt")
                            nc.scalar.activation(
                                tt[:], proj[:],
                                mybir.ActivationFunctionType.Tanh,
                                bias=pd_sb[:, a * BL + b: a * BL + b + 1],
                            )
                            nc.tensor.matmul(
                                sc_ps[32 * sc:32 * sc + 1, :],
                                v_sb[:, a:a + 1], tt[:],
                                tile_position=(0, 32 * sc),
                                start=(a == 0), stop=(a == ACH - 1),
                            )
                        nc.scalar.activation(
                            wuS[32 * sc:32 * sc + 1, :],
                            sc_ps[32 * sc:32 * sc + 1, :],
                            mybir.ActivationFunctionType.Exp)
                    # this batch's natural-layout blocks (consumed by the
                    # context matmuls) ride the same FIFO ring AFTER the et
                    # tiles so they can't front-run the latency-critical stream
                    nc.sync.dma_start(
                        out=encn_tiles[b][:].rearrange("p (c e) -> p c e", c=CCH),
                        in_=encN[b].rearrange("(c p) e -> p c e", p=128))
                    # gather the 4 strided rows into the packed wu tile
                    r = _row(b)
                    nc.gpsimd.dma_start(out=wu_sb[r:r + 1, :],
                                        in_=wuS[0:128:32, :])

            if _STAGE < 3:
                return
            # ---- softmax epilogue (all batches packed on partitions 0..7) ----
            with (
                tc.tile_pool(name="z_ps", bufs=1, space="PSUM") as zp,
                tc.tile_pool(name="tr_ps", bufs=3, space="PSUM") as tp,
                tc.tile_pool(name="ctx_ps", bufs=2, space="PSUM") as cxp,
            ):
                # transpose wu into [128, CCH*BL] bf16, masked, unnormalized;
                # Z accumulates on PE via ones-matmuls over the bf16 chunks
                z_ps = zp.tile([1, BL], F32, tag="z", name="zrow")
                for c in range(CCH):
                    tr = tp.tile([128, BL], F32, tag="tr", name=f"tr{c}")
                    nc.tensor.transpose(
                        tr[:], wu_sb[:, c * 128:(c + 1) * 128], id8_sb[:])
                    sl = slice(c * BL, (c + 1) * BL)
                    nc.vector.tensor_mul(wuT_sb[:, sl], tr[:], maskT_sb[:, sl])
                    nc.tensor.matmul(
                        z_ps[:], ones_colb[:], wuT_sb[:, sl],
                        start=(c == 0), stop=(c == CCH - 1),
                    )
                nc.vector.tensor_copy(z_sb[:], z_ps[:])
                nc.vector.reciprocal(zi_row[:], z_sb[:])
                zb_ps = zp.tile([128, BL], F32, tag="z", name="zb")
                nc.tensor.matmul(zb_ps[:], ones_row[:], zi_row[:],
                                 start=True, stop=True)
                nc.vector.tensor_copy(zi_bcast[:], zb_ps[:])
                zc_ps = zp.tile([BL, 1], F32, tag="z", name="zc")
                nc.tensor.matmul(zc_ps[:], zi_row[:], ones_row[0:1, 0:1],
                                 start=True, stop=True)
                nc.vector.tensor_copy(zi_col[:], zc_ps[:])

                if _STAGE < 4:
                    return
                # weights output: wu * mask * (1/Z), off the critical path
                nc.vector.tensor_mul(wu_sb[:], wu_sb[:], mask_sb[:])
                nc.vector.tensor_scalar_mul(wu_sb[:], wu_sb[:], zi_col[:])
                nc.scalar.dma_start(out=wts_out[:], in_=wu_sb[:])

                if _STAGE < 5:
                    return
                # context matmuls: 4 batches concurrently per column-group
                for g in range(2):
                    cx = cxp.tile([128, E], F32, tag="cx")
                    for c in range(CCH):
                        for j in range(4):
                            b = g * HB + j
                            nc.tensor.matmul(
                                cx[32 * j:32 * j + 1, :],
                                wuT_sb[:, c * BL + g * HB + j:
                                       c * BL + g * HB + j + 1],
                                encn_tiles[b][:, c * E:(c + 1) * E],
                                tile_position=(0, 32 * j),
                                start=(c == 0), stop=(c == CCH - 1),
                            )
                    cxs = es_pool.tile([128, E], F32, tag="cxs")
                    for j in range(4):
                        b = g * HB + j
                        nc.vector.tensor_scalar_mul(
                            cxs[32 * j:32 * j + 1, :],
                            cx[32 * j:32 * j + 1, :],
                            zi_bcast[32 * j:32 * j + 1, b:b + 1])
                        nc.scalar.dma_start(out=ctx_out[b:b + 1, :],
                                            in_=cxs[32 * j:32 * j + 1, :])
    return nc


def _get_nc():
    if "nc" not in _CACHE:
        nc = bacc.Bacc()
        _emit(nc)
        nc.finalize()
        _CACHE["nc"] = nc
    return _CACHE["nc"]


def _make_idq():
    return np.eye(BL, dtype=np.float32)


def kernel(dec_state, enc_outputs, src_mask, W_enc, W_dec, v):
    global LAST_RESULTS
    dec_state = np.asarray(dec_state, dtype=np.float32)
    enc_outputs = np.asarray(enc_outputs, dtype=np.float32)
    src_mask = np.asarray(src_mask)
    W_enc = np.asarray(W_enc, dtype=np.float32)
    W_dec = np.asarray(W_dec, dtype=np.float32)
    v = np.asarray(v, dtype=np.float32)

    encN_full = enc_outputs.astype(BF16)                                  # [B,S,E]
    encT_full = np.ascontiguousarray(enc_outputs.transpose(0, 2, 1)).astype(BF16)
    ds_bf = dec_state.astype(BF16)
    mask_f = src_mask.astype(np.float32)
    We_bf = W_enc.astype(BF16)
    Wd_bf = W_dec.astype(BF16)
    vT = np.ascontiguousarray(v.astype(BF16).reshape(ACH, 128).T)         # [128,ACH]
    idq = _make_idq()

    in_maps = []
    for cid in range(NCORES):
        sl = slice(cid * BL, (cid + 1) * BL)
        mshard = mask_f[sl]                                               # [BL,S]
        mT = np.ascontiguousarray(
            mshard.reshape(BL, CCH, 128).transpose(2, 1, 0).reshape(128, CCH * BL))
        in_maps.append({
            "encT": encT_full[sl],
            "encN": encN_full[sl],
            "dsT": np.ascontiguousarray(ds_bf[sl].T),
            "maskN": mshard,
            "maskT": mT,
            "Wenc": We_bf,
            "Wdec": Wd_bf,
            "vT": vT,
            "idq": idq,
        })

    nc = _get_nc()
    res = run_bass_kernel_spmd(nc, in_maps, core_ids=list(range(NCORES)))
    LAST_RESULTS = res

    ctx = np.concatenate([np.asarray(res.results[c]["ctx"]) for c in range(NCORES)], 0)
    wts = np.concatenate([np.asarray(res.results[c]["wts"]) for c in range(NCORES)], 0)
    return ctx.astype(np.float32), wts.astype(np.float32)


# revision 55
# speedup vs baseline: 1.0198x; 1.0132x over previous
"""Bahdanau attention Trainium2 kernel (8 NeuronCores, data-parallel over batch).

Problem shapes: B=64, S=2048, ENC=DEC=512, ATTN=256 (fp32 inputs).
Strategy:
  - Shard batch across 8 cores (8 batches/core); replicate the small weights.
  - Host pre-casts enc_outputs to bf16 and supplies it in BOTH layouts:
      encT [Bl, E, S]  -> feeds the projection matmul (contract E on partitions)
      encN [Bl, S, E]  -> feeds the context matmul   (contract S on partitions)
    Total HBM read/core = 2 * 16.8MB = 33.6MB ~= the fp32 single-read roofline,
    with zero on-device transpose/cast cost.
  - Per (batch, s-chunk of 512): projT = W_enc^T @ encT (PSUM f32), tanh(+pd bias)
    on ScalarE -> T bf16, score row = v^T @ T (M=1 matmuls col-tiled across the
    4 s-chunks), exp on ScalarE at the col-group lanes, strided-DMA gather into
    a packed wu tile (batch rows at partitions {0..3, 32..35}).
  - Softmax: scores are bounded (|score| <= |v|_1) so no max subtraction needed;
    per batch-half (4 rows), overlapped with the other half's main loop:
    wuT = transpose(wu) * maskT (bf16, unnormalized); Z via PE ones-matmuls
    over wuT; weights = wu * mask * (1/Z);
    context = (wuT^T @ encN) with rows scaled by (1/Z) on the way out of PSUM,
    4 batches concurrently via PE column-group tiling.
"""

import os
import sys

import numpy as np

for _p in ("/opt/trn_rl_repo", "/root/.axon_site/_ro/trn_rl_repo"):
    if os.path.isdir(_p) and _p not in sys.path:
        sys.path.insert(0, _p)

import ml_dtypes

import concourse.bass as bass
import concourse.bacc as bacc
import concourse.mybir as mybir
from concourse import tile
from concourse import bass_utils as _bu
from concourse.bass_utils import run_bass_kernel_spmd


BF16 = ml_dtypes.bfloat16

B, S, E, A = 64, 2048, 512, 256
NCORES = 8
BL = B // NCORES          # batches per core = 8
HB = BL // 2              # half-batch group = 4
SCN = 4                   # s-chunks per row of 512
SCW = S // SCN            # 512
ECH = E // 128            # 4 contraction chunks
ACH = A // 128            # 2 output (attn) chunks
CCH = S // 128            # 16 context contraction chunks

F32 = mybir.dt.float32
BF = mybir.dt.bfloat16

_CACHE = {}
LAST_RESULTS = None

_STAGE = int(os.environ.get("BAHDANAU_STAGE", "5"))


def _row(b):
    """wu/mask row partition for local batch b."""
    return b


def _emit(nc):
    encT = nc.declare_dram_parameter("encT", [BL, E, S], BF, isOutput=False)
    encN = nc.declare_dram_parameter("encN", [BL, S, E], BF, isOutput=False)
    dsT = nc.declare_dram_parameter("dsT", [E, BL], BF, isOutput=False)
    maskN = nc.declare_dram_parameter("maskN", [BL, S], F32, isOutput=False)
    maskT = nc.declare_dram_parameter("maskT", [128, CCH * BL], F32, isOutput=False)
    Wenc = nc.declare_dram_parameter("Wenc", [E, A], BF, isOutput=False)
    Wdec = nc.declare_dram_parameter("Wdec", [E, A], BF, isOutput=False)
    vT = nc.declare_dram_parameter("vT", [128, ACH], BF, isOutput=False)
    idq = nc.declare_dram_parameter("idq", [BL, BL], F32, isOutput=False)
    ctx_out = nc.declare_dram_parameter("ctx", [BL, E], F32, isOutput=True)
    wts_out = nc.declare_dram_parameter("wts", [BL, S], F32, isOutput=True)

    with tile.TileContext(nc) as tc:
        with (
            tc.tile_pool(name="consts", bufs=1) as cp,
            tc.tile_pool(name="encn", bufs=BL) as encn_pool,
            tc.tile_pool(name="et", bufs=6) as et_pool,
            tc.tile_pool(name="tt", bufs=6) as t_pool,
            tc.tile_pool(name="es", bufs=2) as es_pool,
        ):
            # ---- constants / small inputs ----
            wenc_sb = cp.tile([128, ECH * A], BF)      # [:, e*A + a*128 ...]
            wdec_sb = cp.tile([128, ECH * A], BF)
            v_sb = cp.tile([128, ACH], BF)
            ds_sb = cp.tile([128, ECH * BL], BF)
            id8_sb = cp.tile([BL, BL], F32)
            mask_sb = cp.tile([BL, S], F32)
            maskT_sb = cp.tile([128, CCH * BL], F32)
            pd_sb = cp.tile([128, ACH * BL], F32)
            wu_sb = cp.tile([BL, S], F32)
            wuT_sb = cp.tile([128, CCH * BL], BF)
            z_sb = cp.tile([1, BL], F32)
            zi_row = cp.tile([1, BL], F32)
            zi_col = cp.tile([BL, 1], F32)
            zi_bcast = cp.tile([128, BL], F32)
            ones_row = cp.tile([1, 128], F32)
            ones_colb = cp.tile([128, 1], BF)
            nc.vector.memset(ones_row[:], 1.0)
            nc.vector.memset(ones_colb[:], 1.0)

            # constants go on the scalar engine's DMA queue so the sync
            # engine's ring starts with the first encT tile
            nc.scalar.dma_start(
                out=wenc_sb[:].rearrange("p (ec a) -> p ec a", ec=ECH),
                in_=Wenc.rearrange("(ec p) a -> p ec a", p=128))
            nc.scalar.dma_start(
                out=wdec_sb[:].rearrange("p (ec a) -> p ec a", ec=ECH),
                in_=Wdec.rearrange("(ec p) a -> p ec a", p=128))
            nc.scalar.dma_start(
                out=ds_sb[:].rearrange("p (ec b) -> p ec b", ec=ECH),
                in_=dsT.rearrange("(ec p) b -> p ec b", p=128))
            nc.scalar.dma_start(out=v_sb[:], in_=vT[:])
            nc.scalar.dma_start(out=id8_sb[:], in_=idq[:])
            nc.scalar.dma_start(out=mask_sb[:], in_=maskN[:])
            nc.scalar.dma_start(out=maskT_sb[:], in_=maskT[:])

            with tc.tile_pool(name="misc_ps", bufs=1, space="PSUM") as mp:
                # proj_dec^T for all local batches: [128 (a-half), BL] x ACH
                pd_ps = mp.tile([128, ACH * BL], F32)
                for a in range(ACH):
                    for e in range(ECH):
                        nc.tensor.matmul(
                            pd_ps[:, a * BL:(a + 1) * BL],
                            wdec_sb[:, e * A + a * 128: e * A + (a + 1) * 128],
                            ds_sb[:, e * BL:(e + 1) * BL],
                            start=(e == 0), stop=(e == ECH - 1),
                        )
                nc.vector.tensor_copy(pd_sb[:], pd_ps[:])

            if _STAGE < 2:
                return

            encn_tiles = [encn_pool.tile([128, CCH * E], BF, tag="encn",
                                         name=f"ent{i}")
                          for i in range(BL)]

            # ---- main loop: projection + tanh + scores + exp ----
            with (
                tc.tile_pool(name="proj_ps", bufs=6, space="PSUM") as pp,
                tc.tile_pool(name="score_ps", bufs=2, space="PSUM") as sp,
            ):
                for b in range(BL):
                    # one score psum tile per batch; the 4 s-chunks land at
                    # partitions {0,32,64,96} via column-group tiling
                    sc_ps = sp.tile([128, SCW], F32, tag="score")
                    wuS = es_pool.tile([128, SCW], F32, tag="wus")
                    for sc in range(SCN):
                        et = et_pool.tile([128, ECH * SCW], BF, tag="et")
                        et_src = encT[b].rearrange("(ec p) s -> p ec s", p=128)[
                            :, :, sc * SCW:(sc + 1) * SCW]
                        et_dst = et[:].rearrange("p (ec s) -> p ec s", ec=ECH)
                        if b == 0 and sc == 0:
                            # fast start: first tile in per-chunk pieces so the
                            # first matmul fires after ~128KB instead of 1MB
                            for e in range(ECH):
                                nc.sync.dma_start(out=et_dst[:, e:e + 1, :],
                                                  in_=et_src[:, e:e + 1, :])
                        else:
                            nc.sync.dma_start(out=et_dst, in_=et_src)

                        for a in range(ACH):
                            proj = pp.tile([128, SCW], F32, tag="proj")
                            for e in range(ECH):
                                nc.tensor.matmul(
                                    proj[:],
                                    wenc_sb[:, e * A + a * 128: e * A + (a + 1) * 128],
                                    et[:, e * SCW:(e + 1) * SCW],
                                    start=(e == 0), stop=(e == ECH - 1),
                                )
                            tt = t_pool.tile([128, SCW], BF, tag="tt")
                            nc.scalar.activation(
                                tt[:], proj[:],
                                mybir.ActivationFunctionType.Tanh,
                                bias=pd_sb[:, a * BL + b: a * BL + b + 1],
                            )
                            nc.tensor.matmul(
                                sc_ps[32 * sc:32 * sc + 1, :],
                                v_sb[:, a:a + 1], tt[:],
                                tile_position=(0, 32 * sc),
                                start=(a == 0), stop=(a == ACH - 1),
                            )
                        nc.scalar.activation(
                            wuS[32 * sc:32 * sc + 1, :],
                            sc_ps[32 * sc:32 * sc + 1, :],
                            mybir.ActivationFunctionType.Exp)
                    # this batch's natural-layout blocks (consumed by the
                    # context matmuls) ride the same FIFO ring AFTER the et
                    # tiles so they can't front-run the latency-critical stream
                    nc.sync.dma_start(
                        out=encn_tiles[b][:].rearrange("p (c e) -> p c e", c=CCH),
                        in_=encN[b].rearrange("(c p) e -> p c e", p=128))
                    # gather the 4 strided rows into the packed wu tile
                    r = _row(b)
                    nc.gpsimd.dma_start(out=wu_sb[r:r + 1, :],
                                        in_=wuS[0:128:32, :])

            if _STAGE < 3:
                return
            # ---- softmax epilogue (all batches packed on partitions 0..7) ----
            with (
                tc.tile_pool(name="z_ps", bufs=1, space="PSUM") as zp,
                tc.tile_pool(name="tr_ps", bufs=3, space="PSUM") as tp,
                tc.tile_pool(name="ctx_ps", bufs=2, space="PSUM") as cxp,
            ):
                # transpose wu into [128, CCH*BL] bf16, masked, unnormalized;
                # Z accumulates on PE via ones-matmuls over the bf16 chunks
                z_ps = zp.tile([1, BL], F32, tag="z", name="zrow")
                for c in range(CCH):
                    tr = tp.tile([128, BL], F32, tag="tr", name=f"tr{c}")
                    nc.tensor.transpose(
                        tr[:], wu_sb[:, c * 128:(c + 1) * 128], id8_sb[:])
                    sl = slice(c * BL, (c + 1) * BL)
                    nc.vector.tensor_mul(wuT_sb[:, sl], tr[:], maskT_sb[:, sl])
                    nc.tensor.matmul(
                        z_ps[:], ones_colb[:], wuT_sb[:, sl],
                        start=(c == 0), stop=(c == CCH - 1),
                    )
                nc.vector.tensor_copy(z_sb[:], z_ps[:])
                nc.vector.reciprocal(zi_row[:], z_sb[:])
                zb_ps = zp.tile([128, BL], F32, tag="z", name="zb")
                nc.tensor.matmul(zb_ps[:], ones_row[:], zi_row[:],
                                 start=True, stop=True)
                nc.vector.tensor_copy(zi_bcast[:], zb_ps[:])
                zc_ps = zp.tile([BL, 1], F32, tag="z", name="zc")
                nc.tensor.matmul(zc_ps[:], zi_row[:], ones_row[0:1, 0:1],
                                 start=True, stop=True)
                nc.vector.tensor_copy(zi_col[:], zc_ps[:])

                if _STAGE < 4:
                    return
                # weights output: wu * mask * (1/Z), off the critical path
                nc.vector.tensor_mul(wu_sb[:], wu_sb[:], mask_sb[:])
                nc.vector.tensor_scalar_mul(wu_sb[:], wu_sb[:], zi_col[:])
                nc.scalar.dma_start(out=wts_out[:], in_=wu_sb[:])

                if _STAGE < 5:
                    return
                # context matmuls: 4 batches concurrently per column-group
                for g in range(2):
                    cx = cxp.tile([128, E], F32, tag="cx")
                    for c in range(CCH):
                        for j in range(4):
                            b = g * HB + j
                            nc.tensor.matmul(
                                cx[32 * j:32 * j + 1, :],
                                wuT_sb[:, c * BL + g * HB + j:
                                       c * BL + g * HB + j + 1],
                                encn_tiles[b][:, c * E:(c + 1) * E],
                                tile_position=(0, 32 * j),
                                start=(c == 0), stop=(c == CCH - 1),
                            )
                    cxs = es_pool.tile([128, E], F32, tag="cxs")
                    for j in range(4):
                        b = g * HB + j
                        nc.vector.tensor_scalar_mul(
                            cxs[32 * j:32 * j + 1, :],
                            cx[32 * j:32 * j + 1, :],
                            zi_bcast[32 * j:32 * j + 1, b:b + 1])
                        nc.scalar.dma_start(out=ctx_out[b:b + 1, :],
                                            in_=cxs[32 * j:32 * j + 1, :])
    return nc


def _get_nc():
    if "nc" not in _CACHE:
        nc = bacc.Bacc()
        _emit(nc)
        nc.finalize()
        _CACHE["nc"] = nc
    return _CACHE["nc"]


def _make_idq():
    return np.eye(BL, dtype=np.float32)


def kernel(dec_state, enc_outputs, src_mask, W_enc, W_dec, v):
    global LAST_RESULTS
    dec_state = np.asarray(dec_state, dtype=np.float32)
    enc_outputs = np.asarray(enc_outputs, dtype=np.float32)
    src_mask = np.asarray(src_mask)
    W_enc = np.asarray(W_enc, dtype=np.float32)
    W_dec = np.asarray(W_dec, dtype=np.float32)
    v = np.asarray(v, dtype=np.float32)

    encN_full = enc_outputs.astype(BF16)                                  # [B,S,E]
    encT_full = np.ascontiguousarray(enc_outputs.transpose(0, 2, 1)).astype(BF16)
    ds_bf = dec_state.astype(BF16)
    mask_f = src_mask.astype(np.float32)
    We_bf = W_enc.astype(BF16)
    Wd_bf = W_dec.astype(BF16)
    vT = np.ascontiguousarray(v.astype(BF16).reshape(ACH, 128).T)         # [128,ACH]
    idq = _make_idq()

    in_maps = []
    for cid in range(NCORES):
        sl = slice(cid * BL, (cid + 1) * BL)
        mshard = mask_f[sl]                                               # [BL,S]
        mT = np.ascontiguousarray(
            mshard.reshape(BL, CCH, 128).transpose(2, 1, 0).reshape(128, CCH * BL))
        in_maps.append({
            "encT": encT_full[sl],
            "encN": encN_full[sl],
            "dsT": np.ascontiguousarray(ds_bf[sl].T),
            "maskN": mshard,
            "maskT": mT,
            "Wenc": We_bf,
            "Wdec": Wd_bf,
            "vT": vT,
            "idq": idq,
        })

    nc = _get_nc()
    res = run_bass_kernel_spmd(nc, in_maps, core_ids=list(range(NCORES)))
    LAST_RESULTS = res

    ctx = np.concatenate([np.asarray(res.results[c]["ctx"]) for c in range(NCORES)], 0)
    wts = np.concatenate([np.asarray(res.results[c]["wts"]) for c in range(NCORES)], 0)
    return ctx.astype(np.float32), wts.astype(np.float32)


# revision 56
# speedup vs baseline: 1.0461x; 1.0257x over previous
"""Bahdanau attention Trainium2 kernel (8 NeuronCores, data-parallel over batch).

Problem shapes: B=64, S=2048, ENC=DEC=512, ATTN=256 (fp32 inputs).
Strategy:
  - Shard batch across 8 cores (8 batches/core); replicate the small weights.
  - Host pre-casts enc_outputs to bf16 and supplies it in BOTH layouts:
      encT [Bl, E, S]  -> feeds the projection matmul (contract E on partitions)
      encN [Bl, S, E]  -> feeds the context matmul   (contract S on partitions)
    Total HBM read/core = 2 * 16.8MB = 33.6MB ~= the fp32 single-read roofline,
    with zero on-device transpose/cast cost.
  - Per (batch, s-chunk of 512): projT = W_enc^T @ encT (PSUM f32), tanh(+pd bias)
    on ScalarE -> T bf16, score row = v^T @ T (M=1 matmuls col-tiled across the
    4 s-chunks), exp on ScalarE at the col-group lanes, strided-DMA gather into
    a packed wu tile (batch rows at partitions {0..3, 32..35}).
  - Softmax: scores are bounded (|score| <= |v|_1) so no max subtraction needed;
    per batch-half (4 rows), overlapped with the other half's main loop:
    wuT = transpose(wu) * maskT (bf16, unnormalized); Z via PE ones-matmuls
    over wuT; weights = wu * mask * (1/Z);
    context = (wuT^T @ encN) with rows scaled by (1/Z) on the way out of PSUM,
    4 batches concurrently via PE column-group tiling.
"""

import os
import sys

import numpy as np

for _p in ("/opt/trn_rl_repo", "/root/.axon_site/_ro/trn_rl_repo"):
    if os.path.isdir(_p) and _p not in sys.path:
        sys.path.insert(0, _p)

import ml_dtypes

import concourse.bass as bass
import concourse.bacc as bacc
import concourse.mybir as mybir
from concourse import tile
from concourse import bass_utils as _bu
from concourse.bass_utils import run_bass_kernel_spmd


BF16 = ml_dtypes.bfloat16

B, S, E, A = 64, 2048, 512, 256
NCORES = 8
BL = B // NCORES          # batches per core = 8
HB = BL // 2              # half-batch group = 4
SCN = 4                   # s-chunks per row of 512
SCW = S // SCN            # 512
ECH = E // 128            # 4 contraction chunks
ACH = A // 128            # 2 output (attn) chunks
CCH = S // 128            # 16 context contraction chunks

F32 = mybir.dt.float32
BF = mybir.dt.bfloat16

_CACHE = {}
LAST_RESULTS = None

_STAGE = int(os.environ.get("BAHDANAU_STAGE", "5"))


def _row(b):
    """wu/mask row partition for local batch b."""
    return b


def _emit(nc):
    encT = nc.declare_dram_parameter("encT", [BL, E, S], BF, isOutput=False)
    encN = nc.declare_dram_parameter("encN", [BL, S, E], BF, isOutput=False)
    dsT = nc.declare_dram_parameter("dsT", [E, BL], BF, isOutput=False)
    maskN = nc.declare_dram_parameter("maskN", [BL, S], F32, isOutput=False)
    maskT = nc.declare_dram_parameter("maskT", [128, CCH * BL], F32, isOutput=False)
    Wenc = nc.declare_dram_parameter("Wenc", [E, A], BF, isOutput=False)
    Wdec = nc.declare_dram_parameter("Wdec", [E, A], BF, isOutput=False)
    vT = nc.declare_dram_parameter("vT", [128, ACH], BF, isOutput=False)
    idq = nc.declare_dram_parameter("idq", [BL, BL], F32, isOutput=False)
    ctx_out = nc.declare_dram_parameter("ctx", [BL, E], F32, isOutput=True)
    wts_out = nc.declare_dram_parameter("wts", [BL, S], F32, isOutput=True)

    with tile.TileContext(nc) as tc:
        with (
            tc.tile_pool(name="consts", bufs=1) as cp,
            tc.tile_pool(name="encn", bufs=BL) as encn_pool,
            tc.tile_pool(name="et", bufs=6) as et_pool,
            tc.tile_pool(name="tt", bufs=6) as t_pool,
            tc.tile_pool(name="es", bufs=2) as es_pool,
        ):
            # ---- constants / small inputs ----
            wenc_sb = cp.tile([128, ECH * A], BF)      # [:, e*A + a*128 ...]
            wdec_sb = cp.tile([128, ECH * A], BF)
            v_sb = cp.tile([128, ACH], BF)
            ds_sb = cp.tile([128, ECH * BL], BF)
            id8_sb = cp.tile([BL, BL], F32)
            mask_sb = cp.tile([BL, S], F32)
            maskT_sb = cp.tile([128, CCH * BL], F32)
            pd_sb = cp.tile([128, ACH * BL], F32)
            wu_sb = cp.tile([BL, S], F32)
            wuT_sb = cp.tile([128, CCH * BL], BF)
            z_sb = cp.tile([1, BL], F32)
            zi_row = cp.tile([1, BL], F32)
            zi_col = cp.tile([BL, 1], F32)
            zi_bcast = cp.tile([128, BL], F32)
            ones_row = cp.tile([1, 128], F32)
            ones_colb = cp.tile([128, 1], BF)
            nc.vector.memset(ones_row[:], 1.0)
            nc.vector.memset(ones_colb[:], 1.0)

            # constants go on the scalar engine's DMA queue so the sync
            # engine's ring starts with the first encT tile
            nc.scalar.dma_start(
                out=wenc_sb[:].rearrange("p (ec a) -> p ec a", ec=ECH),
                in_=Wenc.rearrange("(ec p) a -> p ec a", p=128))
            nc.scalar.dma_start(
                out=wdec_sb[:].rearrange("p (ec a) -> p ec a", ec=ECH),
                in_=Wdec.rearrange("(ec p) a -> p ec a", p=128))
            nc.scalar.dma_start(
                out=ds_sb[:].rearrange("p (ec b) -> p ec b", ec=ECH),
                in_=dsT.rearrange("(ec p) b -> p ec b", p=128))
            nc.scalar.dma_start(out=v_sb[:], in_=vT[:])
            nc.scalar.dma_start(out=id8_sb[:], in_=idq[:])
            nc.scalar.dma_start(out=mask_sb[:], in_=maskN[:])
            nc.scalar.dma_start(out=maskT_sb[:], in_=maskT[:])

            with tc.tile_pool(name="misc_ps", bufs=1, space="PSUM") as mp:
                # proj_dec^T for all local batches: [128 (a-half), BL] x ACH
                pd_ps = mp.tile([128, ACH * BL], F32)
                for a in range(ACH):
                    for e in range(ECH):
                        nc.tensor.matmul(
                            pd_ps[:, a * BL:(a + 1) * BL],
                            wdec_sb[:, e * A + a * 128: e * A + (a + 1) * 128],
                            ds_sb[:, e * BL:(e + 1) * BL],
                            start=(e == 0), stop=(e == ECH - 1),
                        )
                nc.vector.tensor_copy(pd_sb[:], pd_ps[:])

            if _STAGE < 2:
                return

            encn_tiles = [encn_pool.tile([128, CCH * E], BF, tag="encn",
                                         name=f"ent{i}")
                          for i in range(BL)]

            # ---- main loop: projection + tanh + scores + exp ----
            with (
                tc.tile_pool(name="proj_ps", bufs=6, space="PSUM") as pp,
                tc.tile_pool(name="score_ps", bufs=2, space="PSUM") as sp,
            ):
                for b in range(BL):
                    # one score psum tile per batch; the 4 s-chunks land at
                    # partitions {0,32,64,96} via column-group tiling
                    sc_ps = sp.tile([128, SCW], F32, tag="score")
                    wuS = es_pool.tile([128, SCW], F32, tag="wus")
                    for sc in range(SCN):
                        et = et_pool.tile([128, ECH * SCW], BF, tag="et")
                        et_src = encT[b].rearrange("(ec p) s -> p ec s", p=128)[
                            :, :, sc * SCW:(sc + 1) * SCW]
                        et_dst = et[:].rearrange("p (ec s) -> p ec s", ec=ECH)
                        if b == 0 and sc == 0:
                            # fast start: first tile in per-chunk pieces so the
                            # first matmul fires after ~128KB instead of 1MB
                            for e in range(ECH):
                                nc.sync.dma_start(out=et_dst[:, e:e + 1, :],
                                                  in_=et_src[:, e:e + 1, :])
                        else:
                            nc.sync.dma_start(out=et_dst, in_=et_src)

                        for a in range(ACH):
                            proj = pp.tile([128, SCW], F32, tag="proj")
                            for e in range(ECH):
                                nc.tensor.matmul(
                                    proj[:],
                                    wenc_sb[:, e * A + a * 128: e * A + (a + 1) * 128],
                                    et[:, e * SCW:(e + 1) * SCW],
                                    start=(e == 0), stop=(e == ECH - 1),
                                )
                            tt = t_pool.tile([128, SCW], BF, tag="tt")
                            nc.scalar.activation(
                                tt[:], proj[:],
                                mybir.ActivationFunctionType.Tanh,
                                bias=pd_sb[:, a * BL + b: a * BL + b + 1],
                            )
                            nc.tensor.matmul(
                                sc_ps[32 * sc:32 * sc + 1, :],
                                v_sb[:, a:a + 1], tt[:],
                                tile_position=(0, 32 * sc),
                                start=(a == 0), stop=(a == ACH - 1),
                            )
                        nc.scalar.activation(
                            wuS[32 * sc:32 * sc + 1, :],
                            sc_ps[32 * sc:32 * sc + 1, :],
                            mybir.ActivationFunctionType.Exp)
                    # this batch's natural-layout blocks (consumed by the
                    # context matmuls) go on the scalar engine's separate DMA
                    # ring so the latency-critical et stream has its own queue
                    nc.scalar.dma_start(
                        out=encn_tiles[b][:].rearrange("p (c e) -> p c e", c=CCH),
                        in_=encN[b].rearrange("(c p) e -> p c e", p=128))
                    # gather the 4 strided rows into the packed wu tile
                    r = _row(b)
                    nc.gpsimd.dma_start(out=wu_sb[r:r + 1, :],
                                        in_=wuS[0:128:32, :])

            if _STAGE < 3:
                return
            # ---- softmax epilogue (all batches packed on partitions 0..7) ----
            with (
                tc.tile_pool(name="z_ps", bufs=1, space="PSUM") as zp,
                tc.tile_pool(name="tr_ps", bufs=3, space="PSUM") as tp,
                tc.tile_pool(name="ctx_ps", bufs=2, space="PSUM") as cxp,
            ):
                # transpose wu into [128, CCH*BL] bf16, masked, unnormalized;
                # Z accumulates on PE via ones-matmuls over the bf16 chunks
                z_ps = zp.tile([1, BL], F32, tag="z", name="zrow")
                for c in range(CCH):
                    tr = tp.tile([128, BL], F32, tag="tr", name=f"tr{c}")
                    nc.tensor.transpose(
                        tr[:], wu_sb[:, c * 128:(c + 1) * 128], id8_sb[:])
                    sl = slice(c * BL, (c + 1) * BL)
                    nc.vector.tensor_mul(wuT_sb[:, sl], tr[:], maskT_sb[:, sl])
                    nc.tensor.matmul(
                        z_ps[:], ones_colb[:], wuT_sb[:, sl],
                        start=(c == 0), stop=(c == CCH - 1),
                    )
                nc.vector.tensor_copy(z_sb[:], z_ps[:])
                nc.vector.reciprocal(zi_row[:], z_sb[:])
                zb_ps = zp.tile([128, BL], F32, tag="z", name="zb")
                nc.tensor.matmul(zb_ps[:], ones_row[:], zi_row[:],
                                 start=True, stop=True)
                nc.vector.tensor_copy(zi_bcast[:], zb_ps[:])
                zc_ps = zp.tile([BL, 1], F32, tag="z", name="zc")
                nc.tensor.matmul(zc_ps[:], zi_row[:], ones_row[0:1, 0:1],
                                 start=True, stop=True)
                nc.vector.tensor_copy(zi_col[:], zc_ps[:])

                if _STAGE < 4:
                    return
                # weights output: wu * mask * (1/Z), off the critical path
                nc.vector.tensor_mul(wu_sb[:], wu_sb[:], mask_sb[:])
                nc.vector.tensor_scalar_mul(wu_sb[:], wu_sb[:], zi_col[:])
                nc.scalar.dma_start(out=wts_out[:], in_=wu_sb[:])

                if _STAGE < 5:
                    return
                # context matmuls: 4 batches concurrently per column-group
                for g in range(2):
                    cx = cxp.tile([128, E], F32, tag="cx")
                    for c in range(CCH):
                        for j in range(4):
                            b = g * HB + j
                            nc.tensor.matmul(
                                cx[32 * j:32 * j + 1, :],
                                wuT_sb[:, c * BL + g * HB + j:
                                       c * BL + g * HB + j + 1],
                                encn_tiles[b][:, c * E:(c + 1) * E],
                                tile_position=(0, 32 * j),
                                start=(c == 0), stop=(c == CCH - 1),
                            )
                    cxs = es_pool.tile([128, E], F32, tag="cxs")
                    for j in range(4):
                        b = g * HB + j
                        nc.vector.tensor_scalar_mul(
                            cxs[32 * j:32 * j + 1, :],
                            cx[32 * j:32 * j + 1, :],
                            zi_bcast[32 * j:32 * j + 1, b:b + 1])
                        nc.scalar.dma_start(out=ctx_out[b:b + 1, :],
                                            in_=cxs[32 * j:32 * j + 1, :])
    return nc


def _get_nc():
    if "nc" not in _CACHE:
        nc = bacc.Bacc()
        _emit(nc)
        nc.finalize()
        _CACHE["nc"] = nc
    return _CACHE["nc"]


def _make_idq():
    return np.eye(BL, dtype=np.float32)


def kernel(dec_state, enc_outputs, src_mask, W_enc, W_dec, v):
    global LAST_RESULTS
    dec_state = np.asarray(dec_state, dtype=np.float32)
    enc_outputs = np.asarray(enc_outputs, dtype=np.float32)
    src_mask = np.asarray(src_mask)
    W_enc = np.asarray(W_enc, dtype=np.float32)
    W_dec = np.asarray(W_dec, dtype=np.float32)
    v = np.asarray(v, dtype=np.float32)

    encN_full = enc_outputs.astype(BF16)                                  # [B,S,E]
    encT_full = np.ascontiguousarray(enc_outputs.transpose(0, 2, 1)).astype(BF16)
    ds_bf = dec_state.astype(BF16)
    mask_f = src_mask.astype(np.float32)
    We_bf = W_enc.astype(BF16)
    Wd_bf = W_dec.astype(BF16)
    vT = np.ascontiguousarray(v.astype(BF16).reshape(ACH, 128).T)         # [128,ACH]
    idq = _make_idq()

    in_maps = []
    for cid in range(NCORES):
        sl = slice(cid * BL, (cid + 1) * BL)
        mshard = mask_f[sl]                                               # [BL,S]
        mT = np.ascontiguousarray(
            mshard.reshape(BL, CCH, 128).transpose(2, 1, 0).reshape(128, CCH * BL))
        in_maps.append({
            "encT": encT_full[sl],
            "encN": encN_full[sl],
            "dsT": np.ascontiguousarray(ds_bf[sl].T),
            "maskN": mshard,
            "maskT": mT,
            "Wenc": We_bf,
            "Wdec": Wd_bf,
            "vT": vT,
            "idq": idq,
        })

    nc = _get_nc()
    res = run_bass_kernel_spmd(nc, in_maps, core_ids=list(range(NCORES)))
    LAST_RESULTS = res

    ctx = np.concatenate([np.asarray(res.results[c]["ctx"]) for c in range(NCORES)], 0)
    wts = np.concatenate([np.asarray(res.results[c]["wts"]) for c in range(NCORES)], 0)
    return ctx.astype(np.float32), wts.astype(np.float32)
